# revision 1
# baseline (speedup 1.0000x reference)
"""TT-adapter linear kernel for TRN2, data-parallel over batch on 8 NeuronCores.

Math: out = x @ W.T + b + ALPHA * TT(x), where TT is a tensor-train
factorized linear map (6 small cores).  TT is linear in x, so the module
collapses to a single matmul with a merged weight:

    T  = TT-matrix reconstruction (1024x1024, ~17 MFLOP, folded on host)
    Wc = W + ALPHA * T
    out = x @ Wc.T + b

The 34 GFLOP batched matmul runs on device in bf16 (f32 PSUM accumulation),
one batch element per NeuronCore, no collectives.  Raw bacc (manual
semaphores), measured ~75-78 us on silicon vs a ~55 us pure-TensorE
roofline.

Host layouts (per core, P=128 partitions, contraction dim on partitions):
    xt  bf16 [8, 128, 2048]  xt[d, p, s]   = x[b, s, 128*d + p]
    wt  bf16 [8, 128, 1024]  wt[d, p, o]   = Wc[o, 128*d + p]
    bi  f32  [128, 8]        bi[p, oo]     = b[128*oo + p]
    out f32  [8, 128, 2048]  out[oo, p, s] = result[b, s, 128*oo + p]

Schedule per core (group g = (o, sc), o = g//4, sc = g%4, bank/slot = g%8):
  SP:  (w_d, x_d) DMAs interleaved + bias, then output DMAs g=0..29 gated
       on evictions, final wait for all out-DMA completions.
  PE:  20 short HAM-warm-up matmuls in the preamble/input-latency window, then
       phase 1 = groups 0..7 d-outer staircase (matmuls start as (w_d, x_d)
       arrive), phase 2 = groups 8..31 sequential d-inner, gated on bank
       eviction.  Per-d input semaphores (HWDGE completions are unordered).
  ACT: 32 evictions (PSUM -> SBUF + per-partition bias add) + the last two
       output DMAs shipped directly (skips the SP semaphore hop on the tail).
"""

import numpy as np
import ml_dtypes
from contextlib import ExitStack

import concourse.bass as bass  # noqa: F401
import concourse.mybir as mybir
from concourse import bacc
from concourse.bass_utils import run_bass_kernel_spmd

ALPHA = 16.0
B, S, D = 8, 2048, 1024
P = 128
DO = D // P
OO = D // P
SCH = 512
NS = S // SCH
NG = OO * NS        # 32 groups
NBANK = 8
NSLOT = 8

_NC = None


def _build_nc():
    nc = bacc.Bacc("TRN2", target_bir_lowering=False, debug=False)
    xt = nc.declare_dram_parameter("xt", [DO, P, S], mybir.dt.bfloat16, isOutput=False)
    wt = nc.declare_dram_parameter("wt", [DO, P, D], mybir.dt.bfloat16, isOutput=False)
    bi = nc.declare_dram_parameter("bi", [P, OO], mybir.dt.float32, isOutput=False)
    out = nc.declare_dram_parameter("out", [OO, P, S], mybir.dt.float32, isOutput=True)

    with ExitStack() as ctx:
        block = ctx.enter_context(nc.Block())
        # HWDGE completions on one queue are NOT ordered across DMAs, so a
        # single cumulative input semaphore is racy — use one sem per d-tile
        # (w_d + x_d -> 32) plus one for the bias.
        s_wx = [ctx.enter_context(nc.semaphore(f"s_wx{d}")) for d in range(DO)]
        s_bias = ctx.enter_context(nc.semaphore("s_bias"))
        s_mm = ctx.enter_context(nc.semaphore("s_mm"))
        s_ev = ctx.enter_context(nc.semaphore("s_ev"))
        # per-staging-slot out-DMA completion sems (same ordering concern)
        s_slot = [ctx.enter_context(nc.semaphore(f"s_slot{k}")) for k in range(NSLOT)]
        bias_sb = ctx.enter_context(nc.sbuf_tensor("bias_sb", [P, OO], mybir.dt.float32))
        w_sb = ctx.enter_context(nc.sbuf_tensor("w_sb", [P, DO, D], mybir.dt.bfloat16))
        x_sb = ctx.enter_context(nc.sbuf_tensor("x_sb", [P, DO, S], mybir.dt.bfloat16))
        ot_sb = ctx.enter_context(nc.sbuf_tensor("ot_sb", [P, NSLOT, SCH], mybir.dt.float32))
        ps = [ctx.enter_context(nc.psum_tensor(f"ps{b}", [P, SCH], mybir.dt.float32))
              for b in range(NBANK)]

        @block.sync
        def _(sync: bass.BassEngine):
            for d in range(DO):
                sync.dma_start(out=w_sb[:, d, :], in_=wt[d]).then_inc(s_wx[d], 16)
                sync.dma_start(out=x_sb[:, d, :], in_=xt[d]).then_inc(s_wx[d], 16)
            # bias is only needed by the first eviction (~25us in)
            sync.dma_start(out=bias_sb[:, :], in_=bi[:, :]).then_inc(s_bias, 16)
            for g in range(NG - 2):
                o, sc = g // NS, g % NS
                sync.wait_ge(s_ev, g + 1)
                sync.dma_start(
                    out=out[o, :, sc * SCH:(sc + 1) * SCH],
                    in_=ot_sb[:, g % NSLOT, :],
                ).then_inc(s_slot[g % NSLOT], 16)
            for k in range(NSLOT):
                sync.wait_ge(s_slot[k], 16 * (NG // NSLOT))

        @block.tensor
        def _(tensor: bass.BassEngine):
            # HAM warm-up: dummy matmuls on whatever is in SBUF during the
            # otherwise-idle preamble/input-latency window, so the PE clock
            # gate is at 8/8 when real matmuls start.  Results land in bank 0
            # and are discarded (group 0 re-starts it with start=True).
            for _ in range(20):
                tensor.matmul(
                    ps[0][:, 0:256],
                    w_sb[:, 0, 0:P],
                    x_sb[:, 0, 0:256],
                    start=True,
                    stop=True,
                )
            # phase 1: groups 0..7 on banks 0..7, d-outer staircase
            for i, d in enumerate(range(DO)):
                tensor.wait_ge(s_wx[d], 32)
                for g in range(NBANK):
                    o, sc = g // NS, g % NS
                    mmi = tensor.matmul(
                        ps[g][:, :],
                        w_sb[:, d, o * P:(o + 1) * P],
                        x_sb[:, d, sc * SCH:(sc + 1) * SCH],
                        start=(i == 0),
                        stop=(i == DO - 1),
                    )
                    if i == DO - 1:
                        mmi.then_inc(s_mm, 1)
            # phase 2: groups 8..31 sequential, d-inner
            for g in range(NBANK, NG):
                o, sc = g // NS, g % NS
                tensor.wait_ge(s_ev, g - NBANK + 1)
                for d in range(DO):
                    mmi = tensor.matmul(
                        ps[g % NBANK][:, :],
                        w_sb[:, d, o * P:(o + 1) * P],
                        x_sb[:, d, sc * SCH:(sc + 1) * SCH],
                        start=(d == 0),
                        stop=(d == DO - 1),
                    )
                    if d == DO - 1:
                        mmi.then_inc(s_mm, 1)

        @block.scalar
        def _(scalar: bass.BassEngine):
            scalar.wait_ge(s_bias, 16)
            for g in range(NG):
                o, sc = g // NS, g % NS
                scalar.wait_ge(s_mm, g + 1)
                if g >= NSLOT:
                    scalar.wait_ge(s_slot[g % NSLOT], 16 * (g // NSLOT))
                scalar.add(
                    ot_sb[:, g % NSLOT, :], ps[g % NBANK][:, :], bias_sb[:, o:o + 1]
                ).then_inc(s_ev, 1)
                if g >= NG - 2:
                    # last outputs: ACT (also HWDGE) ships them directly,
                    # skipping the SP semaphore hop on the critical tail
                    scalar.dma_start(
                        out=out[o, :, sc * SCH:(sc + 1) * SCH],
                        in_=ot_sb[:, g % NSLOT, :],
                    ).then_inc(s_slot[g % NSLOT], 16)

    nc.compile()
    return nc


def _get_nc():
    global _NC
    if _NC is None:
        _NC = _build_nc()
    return _NC


def _merged_weight_T(W, b, core0, core1, core2, core3, core4, core5):
    f8 = np.float64
    A = core0[0].astype(f8)
    Bm = np.einsum('ap,pbq->abq', A, core1.astype(f8))
    C = np.einsum('abq,qcr->abcr', Bm, core2.astype(f8))
    Phi = C.transpose(2, 1, 0, 3).reshape(D, 8)
    Dn = np.einsum('paq,qbr->pabr', core3.astype(f8), core4.astype(f8))
    E = np.einsum('pabq,qc->pabc', Dn, core5[:, :, 0].astype(f8))
    Psi = E.reshape(8, D)
    WcT = W.T.astype(f8) + ALPHA * (Phi @ Psi)
    return WcT.astype(np.float32)


def _prep_in_maps(x, W, b, core0, core1, core2, core3, core4, core5):
    WcT = _merged_weight_T(W, b, core0, core1, core2, core3, core4, core5)
    wt = WcT.reshape(DO, P, D).astype(ml_dtypes.bfloat16)
    bi = np.ascontiguousarray(b.reshape(OO, P).T).astype(np.float32)
    in_maps = []
    for bb in range(B):
        xt = x[bb].T.reshape(DO, P, S).astype(ml_dtypes.bfloat16)
        in_maps.append({"xt": xt, "wt": wt, "bi": bi})
    return in_maps


def _gather(results):
    outs = []
    for bb in range(B):
        o = np.asarray(results[bb]["out"])
        outs.append(o.transpose(2, 0, 1).reshape(S, D))
    return np.ascontiguousarray(np.stack(outs)).astype(np.float32)


def run(inputs, **spmd_kwargs):
    inputs = {k: np.asarray(v) for k, v in inputs.items()}
    in_maps = _prep_in_maps(**inputs)
    nc = _get_nc()
    res = run_bass_kernel_spmd(nc, in_maps, core_ids=list(range(B)), **spmd_kwargs)
    return _gather(res.results), res


def kernel(x, W, b, core0, core1, core2, core3, core4, core5):
    out, _ = run(dict(x=x, W=W, b=b, core0=core0, core1=core1, core2=core2,
                      core3=core3, core4=core4, core5=core5))
    return out



# revision 3
# speedup vs baseline: 1.0500x; 1.0500x over previous
"""TT-adapter linear kernel for TRN2, data-parallel over batch on 8 NeuronCores.

Math: out = x @ W.T + b + ALPHA * TT(x).  TT is linear in x, so the module
collapses to a single matmul with a merged weight folded on host:

    Wc = W + ALPHA * T          (T = TT-matrix reconstruction, 1024x1024)
    out = x @ Wc.T + b

The 34 GFLOP batched matmul runs on device in bf16 (f32 PSUM accumulation),
one batch element per NeuronCore, no collectives.  Raw bacc (manual
semaphores).  PE floor is 256 MMs x 216 ns = 55.3 us; the schedule aims to
keep everything else off the critical path.

Host layouts (per core, P=128 partitions, contraction dim on partitions):
    wx  bf16 [8, 128, 2304]   wx[d, p, 0:256]    = Wc[0:256 (o0-1), 128d+p]^T
                              wx[d, p, 256+s]    = x[b, s, 128d+p]
    wb  bf16 [6, 128, 8, 128] wb[oi, p, d, j]    = Wc[128(oi+2)+j, 128d+p]
    bi  f32  [128, 8]         bi[p, oo]          = b[128oo + p]
    out bf16 [8, 128, 2048]   out[oo, p, s]      = result[b, s, 128oo+p]

Schedule per core (group g = 4*o + sc, o = out-tile, sc = 512-col s-chunk):
  SP:  input DMAs in arrival-critical order: wx0 split in two (first MM can
       start after 327 KB), bias, wx1..7, then the six per-o weight slices
       wb[oi] that only phase 2 needs.  Then out-DMAs g=0..29 gated on
       evictions; final wait for the 8 slot-completion sems.
  PE:  12 HAM-warm-up matmuls during the preamble/input-latency window;
       phase 1 = groups 0..7 (o=0,1) d-outer staircase across all 8 PSUM
       banks, gated per-d on wx arrivals; phase 2 = groups 8..31 d-inner
       per group, gated on bank eviction + that o's wb slice.
  ACT: one dummy 8-col activate first so the lazy ACT_TABLE_LOAD (1.3 us)
       runs in the preamble instead of before the first real eviction;
       then 32 evictions (PSUM -> SBUF bf16 + per-partition bias add); the
       last two groups' out-DMAs ship directly from ACT (skips the SP
       semaphore hop on the critical tail).
"""

import numpy as np
import ml_dtypes
from contextlib import ExitStack

import concourse.bass as bass  # noqa: F401
import concourse.mybir as mybir
from concourse import bacc
from concourse.bass_utils import run_bass_kernel_spmd

ALPHA = 16.0
B, S, D = 8, 2048, 1024
P = 128
DO = D // P          # 8 contraction tiles
OO = D // P          # 8 output tiles
SCH = 512
NS = S // SCH        # 4 s-chunks
NG = OO * NS         # 32 groups
NBANK = 8
NSLOT = 8
WXC = 256 + S        # per-d packed row: 256 w-cols (o=0,1) + 2048 x-cols
XOFF = 256

_NC = None


def _build_nc():
    nc = bacc.Bacc("TRN2", target_bir_lowering=False, debug=False)
    wx = nc.declare_dram_parameter("wx", [DO, P, WXC], mybir.dt.bfloat16, isOutput=False)
    wb = nc.declare_dram_parameter("wb", [OO - 2, P, DO, P], mybir.dt.bfloat16, isOutput=False)
    bi = nc.declare_dram_parameter("bi", [P, OO], mybir.dt.float32, isOutput=False)
    out = nc.declare_dram_parameter("out", [OO, P, S], mybir.dt.bfloat16, isOutput=True)

    with ExitStack() as ctx:
        block = ctx.enter_context(nc.Block())
        # HWDGE completions on one queue are NOT ordered across DMAs, so each
        # gating granule gets its own semaphore (each DMA incs by 16).
        s_wxa = ctx.enter_context(nc.semaphore("s_wxa"))          # wx0 first 1280 cols
        s_wx = [ctx.enter_context(nc.semaphore(f"s_wx{d}")) for d in range(DO)]
        s_bias = ctx.enter_context(nc.semaphore("s_bias"))
        s_wb = [ctx.enter_context(nc.semaphore(f"s_wb{i}")) for i in range(OO - 2)]
        s_mm = ctx.enter_context(nc.semaphore("s_mm"))
        s_ev = ctx.enter_context(nc.semaphore("s_ev"))
        s_slot = [ctx.enter_context(nc.semaphore(f"s_slot{k}")) for k in range(NSLOT)]

        wx_sb = ctx.enter_context(nc.sbuf_tensor("wx_sb", [P, DO, WXC], mybir.dt.bfloat16))
        w2_sb = ctx.enter_context(nc.sbuf_tensor("w2_sb", [P, OO - 2, DO, P], mybir.dt.bfloat16))
        bias_sb = ctx.enter_context(nc.sbuf_tensor("bias_sb", [P, OO], mybir.dt.float32))
        ot_sb = ctx.enter_context(nc.sbuf_tensor("ot_sb", [P, NSLOT, SCH], mybir.dt.bfloat16))
        ps = [ctx.enter_context(nc.psum_tensor(f"ps{b}", [P, SCH], mybir.dt.float32))
              for b in range(NBANK)]

        def wsl(o, d):
            if o < 2:
                return wx_sb[:, d, o * P:(o + 1) * P]
            return w2_sb[:, o - 2, d, :]

        def xsl(d, sc):
            return wx_sb[:, d, XOFF + sc * SCH:XOFF + (sc + 1) * SCH]

        @block.sync
        def _(sync: bass.BassEngine):
            # arrival-critical order: first MM needs wx0[:, 0:1280]
            sync.dma_start(out=wx_sb[:, 0, 0:1280], in_=wx[0][:, 0:1280]).then_inc(s_wxa, 16)
            sync.dma_start(out=wx_sb[:, 0, 1280:WXC], in_=wx[0][:, 1280:WXC]).then_inc(s_wx[0], 16)
            sync.dma_start(out=bias_sb[:, :], in_=bi[:, :]).then_inc(s_bias, 16)
            for d in range(1, DO):
                sync.dma_start(out=wx_sb[:, d, :], in_=wx[d]).then_inc(s_wx[d], 16)
            for i in range(OO - 2):
                sync.dma_start(out=w2_sb[:, i, :, :], in_=wb[i]).then_inc(s_wb[i], 16)
            for g in range(NG - 2):
                o, sc = g // NS, g % NS
                sync.wait_ge(s_ev, g + 1)
                sync.dma_start(
                    out=out[o, :, sc * SCH:(sc + 1) * SCH],
                    in_=ot_sb[:, g % NSLOT, :],
                ).then_inc(s_slot[g % NSLOT], 16)
            for k in range(NSLOT):
                sync.wait_ge(s_slot[k], 16 * (NG // NSLOT))

        @block.tensor
        def _(tensor: bass.BassEngine):
            # HAM warm-up on whatever is in SBUF during the otherwise-idle
            # preamble/input-latency window; results discarded (group 0
            # restarts bank 0 with start=True).
            for _ in range(12):
                tensor.matmul(
                    ps[0][:, 0:256],
                    wx_sb[:, 0, 0:P],
                    wx_sb[:, 0, XOFF:XOFF + 256],
                    start=True,
                    stop=True,
                )
            # phase 1: groups 0..7 (o=0,1 x sc=0..3) d-outer staircase
            for d in range(DO):
                tensor.wait_ge(s_wxa if d == 0 else s_wx[d], 16)
                if d == 0:
                    # only x cols 0:1024 have arrived: do sc=0,1 first
                    order = [(0, 0), (0, 1), (1, 0), (1, 1), None,
                             (0, 2), (0, 3), (1, 2), (1, 3)]
                else:
                    order = [(o, sc) for o in range(2) for sc in range(NS)]
                for item in order:
                    if item is None:
                        tensor.wait_ge(s_wx[0], 16)
                        continue
                    o, sc = item
                    g = o * NS + sc
                    mmi = tensor.matmul(
                        ps[g][:, :],
                        wsl(o, d),
                        xsl(d, sc),
                        start=(d == 0),
                        stop=(d == DO - 1),
                    )
                    if d == DO - 1:
                        # d=7 octet runs in group order 0..7, so these incs
                        # arrive in the order the evictions expect
                        mmi.then_inc(s_mm, 1)

            # phase 2: groups 8..31, d-inner per group
            for g in range(NBANK, NG):
                o, sc = g // NS, g % NS
                if sc == 0:
                    tensor.wait_ge(s_wb[o - 2], 16)
                tensor.wait_ge(s_ev, g - NBANK + 1)
                for d in range(DO):
                    mmi = tensor.matmul(
                        ps[g % NBANK][:, :],
                        wsl(o, d),
                        xsl(d, sc),
                        start=(d == 0),
                        stop=(d == DO - 1),
                    )
                    if d == DO - 1:
                        mmi.then_inc(s_mm, 1)

        @block.scalar
        def _(scalar: bass.BassEngine):
            # dummy 8-col activate: pulls the lazy ACT_TABLE_LOAD into the
            # preamble window (it otherwise delays the first real eviction
            # by ~1.3us).  Reads garbage, lands in ot_sb slot 0 which is
            # fully overwritten by eviction 0 before any out-DMA reads it.
            scalar.add(ot_sb[:, 0, 0:8], bias_sb[:, 0:8], 0.0)
            for g in range(NG):
                o, sc = g // NS, g % NS
                if g == 0:
                    scalar.wait_ge(s_bias, 16)
                scalar.wait_ge(s_mm, g + 1)
                if g >= NSLOT:
                    scalar.wait_ge(s_slot[g % NSLOT], 16 * (g // NSLOT))
                scalar.add(
                    ot_sb[:, g % NSLOT, :], ps[g % NBANK][:, :], bias_sb[:, o:o + 1]
                ).then_inc(s_ev, 1)
                if g >= NG - 2:
                    # last outputs: ACT (also HWDGE) ships them directly,
                    # skipping the SP semaphore hop on the critical tail
                    scalar.dma_start(
                        out=out[o, :, sc * SCH:(sc + 1) * SCH],
                        in_=ot_sb[:, g % NSLOT, :],
                    ).then_inc(s_slot[g % NSLOT], 16)

    nc.compile()
    return nc


def _get_nc():
    global _NC
    if _NC is None:
        _NC = _build_nc()
    return _NC


def _merged_weight_T(W, b, core0, core1, core2, core3, core4, core5):
    f8 = np.float64
    A = core0[0].astype(f8)
    Bm = np.einsum('ap,pbq->abq', A, core1.astype(f8))
    C = np.einsum('abq,qcr->abcr', Bm, core2.astype(f8))
    Phi = C.transpose(2, 1, 0, 3).reshape(D, 8)
    Dn = np.einsum('paq,qbr->pabr', core3.astype(f8), core4.astype(f8))
    E = np.einsum('pabq,qc->pabc', Dn, core5[:, :, 0].astype(f8))
    Psi = E.reshape(8, D)
    WcT = W.T.astype(f8) + ALPHA * (Phi @ Psi)
    return WcT.astype(np.float32)


def _prep_in_maps(x, W, b, core0, core1, core2, core3, core4, core5):
    WcT = _merged_weight_T(W, b, core0, core1, core2, core3, core4, core5)
    wt = WcT.reshape(DO, P, D)                     # wt[d, p, :] = Wc.T row
    wa = wt[:, :, 0:2 * P]                         # [DO, P, 256] (o = 0, 1)
    wb_full = np.ascontiguousarray(
        WcT.reshape(DO, P, OO, P).transpose(2, 1, 0, 3)[2:]
    ).astype(ml_dtypes.bfloat16)                   # [6, P, DO, 128]
    bi = np.ascontiguousarray(b.reshape(OO, P).T).astype(np.float32)
    wa16 = wa.astype(ml_dtypes.bfloat16)
    in_maps = []
    for bb in range(B):
        xt = x[bb].T.reshape(DO, P, S).astype(ml_dtypes.bfloat16)
        wxm = np.concatenate([wa16, xt], axis=2)   # [DO, P, 2304]
        in_maps.append({"wx": np.ascontiguousarray(wxm), "wb": wb_full, "bi": bi})
    return in_maps


def _gather(results):
    outs = []
    for bb in range(B):
        o = np.asarray(results[bb]["out"]).astype(np.float32)
        outs.append(o.transpose(2, 0, 1).reshape(S, D))
    return np.ascontiguousarray(np.stack(outs))


def run(inputs, **spmd_kwargs):
    inputs = {k: np.asarray(v) for k, v in inputs.items()}
    in_maps = _prep_in_maps(**inputs)
    nc = _get_nc()
    res = run_bass_kernel_spmd(nc, in_maps, core_ids=list(range(B)), **spmd_kwargs)
    return _gather(res.results), res


def kernel(x, W, b, core0, core1, core2, core3, core4, core5):
    out, _ = run(dict(x=x, W=W, b=b, core0=core0, core1=core1, core2=core2,
                      core3=core3, core4=core4, core5=core5))
    return out
